# revision 5
# baseline (speedup 1.0000x reference)
"""Trainium2 Bass kernel for DigitConvolutionalModel forward pass.

Model: x[B,784] -> 3x3 valid conv (single channel) -> flatten[676]
       -> relu(.@W1+b1) -> relu(.@W2+b2) -> .@W3+b3 -> [B,10]

Strategy (v3):
  - Pure data parallel: batch 32768 sharded 8 ways (4096 rows/core);
    weights replicated.
  - Conv folds into fc1 on the HOST (W1p = C @ W1 as 9 scatter-adds);
    x is transposed to pixel-major and cast to bf16 on the host. The
    device runs a pure bf16 matmul chain (fp32 PSUM accumulation);
    max rel err ~4e-3 vs the 2e-2 gate.
  - All replicated weights ship as ONE packed bf16 dram image (2 DMAs)
    plus one packed f32 bias image, issued after the first x tile so
    compute starts as early as possible.
  - The per-tile stages are software-pipelined 4 deep in emission
    order: iteration t runs fc1(t), fc2(t-1), fc3(t-2), store(t-3).
    Engines execute in order, so this gives every cross-engine
    dependency a full iteration (~6us) of slack and the PE streams
    back-to-back at its ~213ns/512-row bf16 issue rate.
  - Bias+ReLU PSUM evictions alternate between ScalarE and VectorE
    (GpSimd cannot read PSUM).
"""

import sys

for _p in (
    "/opt/trn_rl_repo",
    "/root/.axon_site",
    "/root/.axon_site/_ro/trn_rl_repo",
    "/root/.axon_site/_ro/pypackages",
):
    if _p not in sys.path:
        sys.path.append(_p)

from contextlib import ExitStack

import numpy as np
import ml_dtypes

import concourse.bass as bass
import concourse.tile as tile
from concourse import mybir
from concourse.bass_utils import run_bass_kernel_spmd
from concourse.masks import make_identity

F32 = mybir.dt.float32
BF16 = mybir.dt.bfloat16
AFT = mybir.ActivationFunctionType
ALU = mybir.AluOpType

B_FULL = 32768
N_CORES = 8
B_CORE = B_FULL // N_CORES  # 4096
IMG = 28
OHW = 26
FLAT = OHW * OHW  # 676
NPIX = IMG * IMG  # 784
HID = 300
NCLS = 10

BT = 512  # batch tile (matmul moving free dim)
NBT = B_CORE // BT  # 8
NBC = BT // 128  # 4

PC = 112  # pixel k-chunk width (784 = 7 x 112)
NPC = NPIX // PC  # 7
H_CH = [(s, min(128, HID - s)) for s in range(0, HID, 128)]  # (0,128),(128,128),(256,44)

# packed weight image columns (bf16): w1s | w2 chunks | w3 chunks
W1_COLS = NPC * HID  # 2100
W2_OFF = W1_COLS
W3_OFF = W2_OFF + 3 * HID  # 3000
WP_COLS = W3_OFF + 3 * NCLS  # 3030


def _legalize_single_wait(nc):
    """This walrus build accepts only one sync-wait per instruction; move
    extra waits onto NoOps inserted just before, on the same engine."""
    n = 0
    for fn in nc.m.functions:
        for bb in fn.blocks:
            new_insts = []
            for inst in bb.instructions:
                si = inst.sync_info
                if si is not None and si.on_wait and len(si.on_wait) > 1:
                    waits = list(si.on_wait)
                    for w in waits[:-1]:
                        nop = mybir.InstNoOp(
                            name=f"{inst.name}-w{n}",
                            sync_info=mybir.SyncInfo(on_wait=[w], on_update=[]),
                            bass_nofuse=True,
                            engine=inst.engine,
                        )
                        n += 1
                        nc.register_instruction(nop, overwrite=True)
                        new_insts.append(nop)
                    inst.sync_info = mybir.SyncInfo(
                        on_wait=[waits[-1]], on_update=list(si.on_update)
                    )
                new_insts.append(inst)
            bb.instructions = new_insts
    return n


def _emit(ctx: ExitStack, tc: tile.TileContext, xt, wp, bp, out):
    nc = tc.nc

    const = ctx.enter_context(tc.tile_pool(name="const", bufs=1))
    ps1p = ctx.enter_context(tc.tile_pool(name="ps1p", bufs=3, space="PSUM"))
    ps2p = ctx.enter_context(tc.tile_pool(name="ps2p", bufs=3, space="PSUM"))
    ps3p = ctx.enter_context(tc.tile_pool(name="ps3p", bufs=1, space="PSUM"))
    psop = ctx.enter_context(tc.tile_pool(name="psop", bufs=1, space="PSUM"))
    xtp = ctx.enter_context(tc.tile_pool(name="xtp", bufs=4))
    hp_ = ctx.enter_context(tc.tile_pool(name="hp", bufs=2))
    op_ = ctx.enter_context(tc.tile_pool(name="op", bufs=2))
    obp = ctx.enter_context(tc.tile_pool(name="obp", bufs=4))

    def load_x(t):
        """One 3D DMA: xt dram [784, 4096] cols [t*512,(t+1)*512) -> SBUF
        [112, 7, 512] bf16 (pixel chunk-major)."""
        xtile = xtp.tile([PC, NPC * BT], BF16, name="xt", tag="xt")
        nc.sync.dma_start(
            xtile[:, :].rearrange("p (c n) -> p c n", c=NPC),
            xt[:, t * BT : (t + 1) * BT].rearrange("(c p) n -> p c n", p=PC),
        )
        return xtile

    # x tile 0 first (fc1(0) blocks on it), then weights, then more x.
    xts = {0: load_x(0)}

    wsb = const.tile([128, WP_COLS], BF16, name="wsb")
    nc.sync.dma_start(wsb[:, 0:W1_COLS], wp[:, 0:W1_COLS])
    bsb = const.tile([128, 7], F32, name="bsb")
    nc.sync.dma_start(bsb[:, :], bp[:, :])
    nc.sync.dma_start(wsb[:, W1_COLS:WP_COLS], wp[:, W1_COLS:WP_COLS])

    for t in (1, 2):
        xts[t] = load_x(t)

    ident = const.tile([128, 128], F32, name="ident")
    make_identity(nc, ident)

    def w1_ap(c, h0, hp):
        return wsb[0:PC, c * HID + h0 : c * HID + h0 + hp]

    def w2_ap(hc, hp, g0, gp):
        return wsb[0:hp, W2_OFF + hc * HID + g0 : W2_OFF + hc * HID + g0 + gp]

    def w3_ap(hc, hp):
        return wsb[0:hp, W3_OFF + hc * NCLS : W3_OFF + (hc + 1) * NCLS]

    def bias_ap(col, hp):
        return bsb[0:hp, col : col + 1]

    h1s = {}
    h2s = {}
    obs = {}

    def fc1(t):
        xtile = xts.pop(t)
        h1 = []
        for hc, (h0, hp) in enumerate(H_CH):
            ps = ps1p.tile([128, BT], F32, name="ps1", tag="ps1")
            for c in range(NPC):
                nc.tensor.matmul(
                    ps[0:hp, 0:BT],
                    w1_ap(c, h0, hp),
                    xtile[:, c * BT : (c + 1) * BT],
                    start=(c == 0),
                    stop=(c == NPC - 1),
                )
            h = hp_.tile([hp, BT], BF16, name=f"h1_{hc}", tag=f"h1_{hc}")
            if hc in (0, 2):
                nc.scalar.activation(
                    h[:, :], ps[0:hp, 0:BT], AFT.Relu, bias=bias_ap(hc, hp)
                )
            else:
                nc.vector.tensor_scalar(
                    h[:, :], ps[0:hp, 0:BT], bias_ap(hc, hp), 0.0,
                    ALU.add, ALU.max,
                )
            h1.append(h)
        h1s[t] = h1
        if t + 3 < NBT:
            xts[t + 3] = load_x(t + 3)

    def fc2(t):
        h1 = h1s.pop(t)
        ps2 = [
            ps2p.tile([128, BT], F32, name=f"ps2_{g}", tag="ps2")
            for g in range(len(H_CH))
        ]
        for hc, (h0, hp) in enumerate(H_CH):
            for g, (g0, gp) in enumerate(H_CH):
                nc.tensor.matmul(
                    ps2[g][0:gp, 0:BT],
                    w2_ap(hc, hp, g0, gp),
                    h1[hc][:, :],
                    start=(hc == 0),
                    stop=(hc == len(H_CH) - 1),
                )
        h2 = []
        for g, (g0, gp) in enumerate(H_CH):
            h = hp_.tile([gp, BT], BF16, name=f"h2_{g}", tag=f"h2_{g}")
            if g in (0, 2):
                nc.vector.tensor_scalar(
                    h[:, :], ps2[g][0:gp, 0:BT], bias_ap(3 + g, gp), 0.0,
                    ALU.add, ALU.max,
                )
            else:
                nc.scalar.activation(
                    h[:, :], ps2[g][0:gp, 0:BT], AFT.Relu, bias=bias_ap(3 + g, gp)
                )
            h2.append(h)
        h2s[t] = h2

    def fc3(t):
        h2 = h2s.pop(t)
        ps3 = ps3p.tile([NCLS, BT], F32, name="ps3", tag="ps3")
        for hc, (h0, hp) in enumerate(H_CH):
            nc.tensor.matmul(
                ps3[0:NCLS, 0:BT],
                w3_ap(hc, hp),
                h2[hc][:, :],
                start=(hc == 0),
                stop=(hc == len(H_CH) - 1),
            )
        ob = op_.tile([NCLS, BT], F32, name="ob", tag="ob")
        nc.scalar.activation(
            ob[:, :], ps3[0:NCLS, 0:BT], AFT.Identity, bias=bias_ap(6, NCLS)
        )
        obs[t] = ob

    def store(t):
        ob = obs.pop(t)
        r0 = t * BT
        po = psop.tile([128, NBC * NCLS], F32, name="po", tag="po")
        for bc in range(NBC):
            nc.tensor.transpose(
                po[0:128, bc * NCLS : (bc + 1) * NCLS],
                ob[:, bc * 128 : (bc + 1) * 128],
                ident[0:NCLS, 0:NCLS],
            )
        os_ = obp.tile([128, NBC * NCLS], F32, name="os", tag="os")
        nc.vector.tensor_copy(os_[:, :], po[0:128, 0 : NBC * NCLS])
        nc.sync.dma_start(
            out[r0 : r0 + BT, :].rearrange("(bc b) c -> b bc c", bc=NBC),
            os_[:, :].rearrange("b (bc c) -> b bc c", bc=NBC),
        )

    for it in range(NBT + 3):
        if it < NBT:
            fc1(it)
        if 0 <= it - 1 < NBT:
            fc2(it - 1)
        if 0 <= it - 2 < NBT:
            fc3(it - 2)
        if 0 <= it - 3 < NBT:
            store(it - 3)


def _build_w1p(conv_w: np.ndarray, W1: np.ndarray) -> np.ndarray:
    """Fold the 3x3 valid conv into fc1: W1p[p, :] = sum over taps landing
    on pixel p of conv_w[dy,dx] * W1[q(p,dy,dx), :]. Zero-FLOP scatter-add."""
    w1p = np.zeros((NPIX, HID), np.float32)
    oi = np.arange(OHW)
    oj = np.arange(OHW)
    q = (oi[:, None] * OHW + oj[None, :]).ravel()
    for dy in range(3):
        for dx in range(3):
            p = ((oi[:, None] + dy) * IMG + (oj[None, :] + dx)).ravel()
            np.add.at(w1p, p, conv_w[dy, dx] * W1[q, :])
    return w1p


_NC_CACHE: list = []


def _get_nc():
    if _NC_CACHE:
        return _NC_CACHE[0]
    nc = bass.Bass("TRN2", target_bir_lowering=False, debug=False)
    xt = nc.dram_tensor("xt", [NPIX, B_CORE], BF16, kind="ExternalInput").ap()
    wp = nc.dram_tensor("wp", [128, WP_COLS], BF16, kind="ExternalInput").ap()
    bp = nc.dram_tensor("bp", [128, 7], F32, kind="ExternalInput").ap()
    out = nc.dram_tensor("out", [B_CORE, NCLS], F32, kind="ExternalOutput").ap()
    with tile.TileContext(nc) as tc:
        with ExitStack() as ctx:
            _emit(ctx, tc, xt, wp, bp, out)
    _legalize_single_wait(nc)
    _NC_CACHE.append(nc)
    return nc


def _pack_weights(inputs: dict) -> tuple:
    bf = ml_dtypes.bfloat16
    w1p = _build_w1p(
        np.asarray(inputs["conv_w"], dtype=np.float32),
        np.asarray(inputs["W1"], dtype=np.float32),
    )
    w2 = np.asarray(inputs["W2"], np.float32)
    w3 = np.asarray(inputs["W3"], np.float32)
    wp = np.zeros((128, WP_COLS), bf)
    # w1s: [112, 7, 300] pixel chunk-major
    wp[0:PC, 0:W1_COLS] = (
        w1p.reshape(NPC, PC, HID).transpose(1, 0, 2).reshape(PC, W1_COLS).astype(bf)
    )
    for hc, (h0, hp) in enumerate(H_CH):
        wp[0:hp, W2_OFF + hc * HID : W2_OFF + (hc + 1) * HID] = w2[
            h0 : h0 + hp, :
        ].astype(bf)
        wp[0:hp, W3_OFF + hc * NCLS : W3_OFF + (hc + 1) * NCLS] = w3[
            h0 : h0 + hp, :
        ].astype(bf)
    bpk = np.zeros((128, 7), np.float32)
    b1 = np.asarray(inputs["b1"], np.float32)
    b2 = np.asarray(inputs["b2"], np.float32)
    b3 = np.asarray(inputs["b3"], np.float32)
    for hc, (h0, hp) in enumerate(H_CH):
        bpk[0:hp, hc] = b1[h0 : h0 + hp]
        bpk[0:hp, 3 + hc] = b2[h0 : h0 + hp]
    bpk[0:NCLS, 6] = b3
    return wp, bpk


def _in_maps(inputs: dict) -> list:
    x = np.asarray(inputs["x"], dtype=np.float32)
    assert x.shape == (B_FULL, NPIX), x.shape
    wp, bpk = _pack_weights(inputs)
    bf = ml_dtypes.bfloat16
    common = {"wp": wp, "bp": bpk}
    xr = x.reshape(N_CORES, B_CORE, NPIX)
    return [
        {"xt": np.ascontiguousarray(xr[c].T.astype(bf)), **common}
        for c in range(N_CORES)
    ]


def kernel(**inputs) -> np.ndarray:
    nc = _get_nc()
    res = run_bass_kernel_spmd(nc, _in_maps(inputs), list(range(N_CORES)))
    return np.concatenate(
        [res.results[c]["out"] for c in range(N_CORES)], axis=0
    )


if __name__ == "__main__":
    rng = np.random.default_rng(0)
    ins = {
        "x": rng.standard_normal((B_FULL, NPIX), dtype=np.float32),
        "conv_w": rng.standard_normal((3, 3), dtype=np.float32) * 0.1,
        "W1": rng.standard_normal((FLAT, HID), dtype=np.float32) * 0.04,
        "b1": np.zeros(HID, np.float32),
        "W2": rng.standard_normal((HID, HID), dtype=np.float32) * 0.06,
        "b2": np.zeros(HID, np.float32),
        "W3": rng.standard_normal((HID, NCLS), dtype=np.float32) * 0.06,
        "b3": np.zeros(NCLS, np.float32),
    }
    y = kernel(**ins)
    from numpy.lib.stride_tricks import sliding_window_view

    img = ins["x"].reshape(-1, IMG, IMG)
    win = sliding_window_view(img, (3, 3), axis=(1, 2))
    conv = np.einsum("bijkl,kl->bij", win, ins["conv_w"]).reshape(-1, FLAT)
    h = np.maximum(conv @ ins["W1"] + ins["b1"], 0)
    h = np.maximum(h @ ins["W2"] + ins["b2"], 0)
    ref = h @ ins["W3"] + ins["b3"]
    err = np.abs(y - ref).max() / (np.abs(ref).max() + 1e-9)
    print("max rel err vs numpy:", err)


# revision 6
# speedup vs baseline: 1.0068x; 1.0068x over previous
"""Trainium2 Bass kernel for DigitConvolutionalModel forward pass.

Model: x[B,784] -> 3x3 valid conv (single channel) -> flatten[676]
       -> relu(.@W1+b1) -> relu(.@W2+b2) -> .@W3+b3 -> [B,10]

Strategy (v3):
  - Pure data parallel: batch 32768 sharded 8 ways (4096 rows/core);
    weights replicated.
  - Conv folds into fc1 on the HOST (W1p = C @ W1 as 9 scatter-adds);
    x is transposed to pixel-major and cast to bf16 on the host. The
    device runs a pure bf16 matmul chain (fp32 PSUM accumulation);
    max rel err ~4e-3 vs the 2e-2 gate.
  - All replicated weights ship as ONE packed bf16 dram image (2 DMAs)
    plus one packed f32 bias image, issued after the first x tile so
    compute starts as early as possible.
  - The per-tile stages are software-pipelined 4 deep in emission
    order: iteration t runs fc1(t), fc2(t-1), fc3(t-2), store(t-3).
    Engines execute in order, so this gives every cross-engine
    dependency a full iteration (~6us) of slack and the PE streams
    back-to-back at its ~213ns/512-row bf16 issue rate.
  - Bias+ReLU PSUM evictions alternate between ScalarE and VectorE
    (GpSimd cannot read PSUM).
"""

import sys

for _p in (
    "/opt/trn_rl_repo",
    "/root/.axon_site",
    "/root/.axon_site/_ro/trn_rl_repo",
    "/root/.axon_site/_ro/pypackages",
):
    if _p not in sys.path:
        sys.path.append(_p)

from contextlib import ExitStack

import numpy as np
import ml_dtypes

import concourse.bass as bass
import concourse.tile as tile
from concourse import mybir
from concourse.bass_utils import run_bass_kernel_spmd
from concourse.masks import make_identity

F32 = mybir.dt.float32
BF16 = mybir.dt.bfloat16
AFT = mybir.ActivationFunctionType
ALU = mybir.AluOpType

B_FULL = 32768
N_CORES = 8
B_CORE = B_FULL // N_CORES  # 4096
IMG = 28
OHW = 26
FLAT = OHW * OHW  # 676
NPIX = IMG * IMG  # 784
HID = 300
NCLS = 10

BT = 512  # batch tile (matmul moving free dim)
NBT = B_CORE // BT  # 8
NBC = BT // 128  # 4

PC = 112  # pixel k-chunk width (784 = 7 x 112)
NPC = NPIX // PC  # 7
H_CH = [(s, min(128, HID - s)) for s in range(0, HID, 128)]  # (0,128),(128,128),(256,44)

# packed weight image columns (bf16): w1s | w2 chunks | w3 chunks
W1_COLS = NPC * HID  # 2100
W2_OFF = W1_COLS
W3_OFF = W2_OFF + 3 * HID  # 3000
WP_COLS = W3_OFF + 3 * NCLS  # 3030


def _legalize_single_wait(nc):
    """This walrus build accepts only one sync-wait per instruction; move
    extra waits onto NoOps inserted just before, on the same engine."""
    n = 0
    for fn in nc.m.functions:
        for bb in fn.blocks:
            new_insts = []
            for inst in bb.instructions:
                si = inst.sync_info
                if si is not None and si.on_wait and len(si.on_wait) > 1:
                    waits = list(si.on_wait)
                    for w in waits[:-1]:
                        nop = mybir.InstNoOp(
                            name=f"{inst.name}-w{n}",
                            sync_info=mybir.SyncInfo(on_wait=[w], on_update=[]),
                            bass_nofuse=True,
                            engine=inst.engine,
                        )
                        n += 1
                        nc.register_instruction(nop, overwrite=True)
                        new_insts.append(nop)
                    inst.sync_info = mybir.SyncInfo(
                        on_wait=[waits[-1]], on_update=list(si.on_update)
                    )
                new_insts.append(inst)
            bb.instructions = new_insts
    return n


def _emit(ctx: ExitStack, tc: tile.TileContext, xt, wp, bp, out):
    nc = tc.nc

    const = ctx.enter_context(tc.tile_pool(name="const", bufs=1))
    ps1p = ctx.enter_context(tc.tile_pool(name="ps1p", bufs=3, space="PSUM"))
    ps2p = ctx.enter_context(tc.tile_pool(name="ps2p", bufs=3, space="PSUM"))
    ps3p = ctx.enter_context(tc.tile_pool(name="ps3p", bufs=1, space="PSUM"))
    psop = ctx.enter_context(tc.tile_pool(name="psop", bufs=1, space="PSUM"))
    xtp = ctx.enter_context(tc.tile_pool(name="xtp", bufs=4))
    hp_ = ctx.enter_context(tc.tile_pool(name="hp", bufs=2))
    op_ = ctx.enter_context(tc.tile_pool(name="op", bufs=2))
    obp = ctx.enter_context(tc.tile_pool(name="obp", bufs=4))

    def load_x(t):
        """One 3D DMA: xt dram [784, 4096] cols [t*512,(t+1)*512) -> SBUF
        [112, 7, 512] bf16 (pixel chunk-major)."""
        xtile = xtp.tile([PC, NPC * BT], BF16, name="xt", tag="xt")
        nc.sync.dma_start(
            xtile[:, :].rearrange("p (c n) -> p c n", c=NPC),
            xt[:, t * BT : (t + 1) * BT].rearrange("(c p) n -> p c n", p=PC),
        )
        return xtile

    # x tile 0 first (fc1(0) blocks on it), then weights, then more x.
    xts = {0: load_x(0)}

    wsb = const.tile([128, WP_COLS], BF16, name="wsb")
    nc.sync.dma_start(wsb[:, 0:W1_COLS], wp[:, 0:W1_COLS])
    bsb = const.tile([128, 7], F32, name="bsb")
    nc.sync.dma_start(bsb[:, :], bp[:, :])
    nc.sync.dma_start(wsb[:, W1_COLS:WP_COLS], wp[:, W1_COLS:WP_COLS])

    for t in (1, 2):
        xts[t] = load_x(t)

    ident = const.tile([128, 128], F32, name="ident")
    make_identity(nc, ident)

    def w1_ap(c, h0, hp):
        return wsb[0:PC, c * HID + h0 : c * HID + h0 + hp]

    def w2_ap(hc, hp, g0, gp):
        return wsb[0:hp, W2_OFF + hc * HID + g0 : W2_OFF + hc * HID + g0 + gp]

    def w3_ap(hc, hp):
        return wsb[0:hp, W3_OFF + hc * NCLS : W3_OFF + (hc + 1) * NCLS]

    def bias_ap(col, hp):
        return bsb[0:hp, col : col + 1]

    h1s = {}
    h2s = {}
    obs = {}

    def fc1(t):
        xtile = xts.pop(t)
        h1 = []
        for hc, (h0, hp) in enumerate(H_CH):
            ps = ps1p.tile([128, BT], F32, name="ps1", tag="ps1")
            for c in range(NPC):
                nc.tensor.matmul(
                    ps[0:hp, 0:BT],
                    w1_ap(c, h0, hp),
                    xtile[:, c * BT : (c + 1) * BT],
                    start=(c == 0),
                    stop=(c == NPC - 1),
                )
            h = hp_.tile([hp, BT], BF16, name=f"h1_{hc}", tag=f"h1_{hc}")
            if hc in (0, 2):
                nc.scalar.activation(
                    h[:, :], ps[0:hp, 0:BT], AFT.Relu, bias=bias_ap(hc, hp)
                )
            else:
                nc.vector.tensor_scalar(
                    h[:, :], ps[0:hp, 0:BT], bias_ap(hc, hp), 0.0,
                    ALU.add, ALU.max,
                )
            h1.append(h)
        h1s[t] = h1
        if t + 3 < NBT:
            xts[t + 3] = load_x(t + 3)

    def fc2(t):
        # g-outer / k-inner: consecutive matmuls share a PSUM bank (bank
        # switches between back-to-back matmuls cost ~95ns on HW). All h1
        # chunks are ready a full iteration ahead, so k-inner never stalls.
        h1 = h1s.pop(t)
        h2 = []
        for g, (g0, gp) in enumerate(H_CH):
            ps = ps2p.tile([128, BT], F32, name=f"ps2_{g}", tag="ps2")
            for hc, (h0, hp) in enumerate(H_CH):
                nc.tensor.matmul(
                    ps[0:gp, 0:BT],
                    w2_ap(hc, hp, g0, gp),
                    h1[hc][:, :],
                    start=(hc == 0),
                    stop=(hc == len(H_CH) - 1),
                )
            h = hp_.tile([gp, BT], BF16, name=f"h2_{g}", tag=f"h2_{g}")
            if g in (0, 2):
                nc.vector.tensor_scalar(
                    h[:, :], ps[0:gp, 0:BT], bias_ap(3 + g, gp), 0.0,
                    ALU.add, ALU.max,
                )
            else:
                nc.scalar.activation(
                    h[:, :], ps[0:gp, 0:BT], AFT.Relu, bias=bias_ap(3 + g, gp)
                )
            h2.append(h)
        h2s[t] = h2

    def fc3(t):
        h2 = h2s.pop(t)
        ps3 = ps3p.tile([NCLS, BT], F32, name="ps3", tag="ps3")
        for hc, (h0, hp) in enumerate(H_CH):
            nc.tensor.matmul(
                ps3[0:NCLS, 0:BT],
                w3_ap(hc, hp),
                h2[hc][:, :],
                start=(hc == 0),
                stop=(hc == len(H_CH) - 1),
            )
        ob = op_.tile([NCLS, BT], F32, name="ob", tag="ob")
        nc.scalar.activation(
            ob[:, :], ps3[0:NCLS, 0:BT], AFT.Identity, bias=bias_ap(6, NCLS)
        )
        obs[t] = ob

    def store(t):
        ob = obs.pop(t)
        r0 = t * BT
        po = psop.tile([128, NBC * NCLS], F32, name="po", tag="po")
        for bc in range(NBC):
            nc.tensor.transpose(
                po[0:128, bc * NCLS : (bc + 1) * NCLS],
                ob[:, bc * 128 : (bc + 1) * 128],
                ident[0:NCLS, 0:NCLS],
            )
        os_ = obp.tile([128, NBC * NCLS], F32, name="os", tag="os")
        nc.vector.tensor_copy(os_[:, :], po[0:128, 0 : NBC * NCLS])
        nc.sync.dma_start(
            out[r0 : r0 + BT, :].rearrange("(bc b) c -> b bc c", bc=NBC),
            os_[:, :].rearrange("b (bc c) -> b bc c", bc=NBC),
        )

    for it in range(NBT + 3):
        if it < NBT:
            fc1(it)
        if 0 <= it - 1 < NBT:
            fc2(it - 1)
        if 0 <= it - 2 < NBT:
            fc3(it - 2)
        if 0 <= it - 3 < NBT:
            store(it - 3)


def _build_w1p(conv_w: np.ndarray, W1: np.ndarray) -> np.ndarray:
    """Fold the 3x3 valid conv into fc1: W1p[p, :] = sum over taps landing
    on pixel p of conv_w[dy,dx] * W1[q(p,dy,dx), :]. Zero-FLOP scatter-add."""
    w1p = np.zeros((NPIX, HID), np.float32)
    oi = np.arange(OHW)
    oj = np.arange(OHW)
    q = (oi[:, None] * OHW + oj[None, :]).ravel()
    for dy in range(3):
        for dx in range(3):
            p = ((oi[:, None] + dy) * IMG + (oj[None, :] + dx)).ravel()
            np.add.at(w1p, p, conv_w[dy, dx] * W1[q, :])
    return w1p


_NC_CACHE: list = []


def _get_nc():
    if _NC_CACHE:
        return _NC_CACHE[0]
    nc = bass.Bass("TRN2", target_bir_lowering=False, debug=False)
    xt = nc.dram_tensor("xt", [NPIX, B_CORE], BF16, kind="ExternalInput").ap()
    wp = nc.dram_tensor("wp", [128, WP_COLS], BF16, kind="ExternalInput").ap()
    bp = nc.dram_tensor("bp", [128, 7], F32, kind="ExternalInput").ap()
    out = nc.dram_tensor("out", [B_CORE, NCLS], F32, kind="ExternalOutput").ap()
    with tile.TileContext(nc) as tc:
        with ExitStack() as ctx:
            _emit(ctx, tc, xt, wp, bp, out)
    _legalize_single_wait(nc)
    _NC_CACHE.append(nc)
    return nc


def _pack_weights(inputs: dict) -> tuple:
    bf = ml_dtypes.bfloat16
    w1p = _build_w1p(
        np.asarray(inputs["conv_w"], dtype=np.float32),
        np.asarray(inputs["W1"], dtype=np.float32),
    )
    w2 = np.asarray(inputs["W2"], np.float32)
    w3 = np.asarray(inputs["W3"], np.float32)
    wp = np.zeros((128, WP_COLS), bf)
    # w1s: [112, 7, 300] pixel chunk-major
    wp[0:PC, 0:W1_COLS] = (
        w1p.reshape(NPC, PC, HID).transpose(1, 0, 2).reshape(PC, W1_COLS).astype(bf)
    )
    for hc, (h0, hp) in enumerate(H_CH):
        wp[0:hp, W2_OFF + hc * HID : W2_OFF + (hc + 1) * HID] = w2[
            h0 : h0 + hp, :
        ].astype(bf)
        wp[0:hp, W3_OFF + hc * NCLS : W3_OFF + (hc + 1) * NCLS] = w3[
            h0 : h0 + hp, :
        ].astype(bf)
    bpk = np.zeros((128, 7), np.float32)
    b1 = np.asarray(inputs["b1"], np.float32)
    b2 = np.asarray(inputs["b2"], np.float32)
    b3 = np.asarray(inputs["b3"], np.float32)
    for hc, (h0, hp) in enumerate(H_CH):
        bpk[0:hp, hc] = b1[h0 : h0 + hp]
        bpk[0:hp, 3 + hc] = b2[h0 : h0 + hp]
    bpk[0:NCLS, 6] = b3
    return wp, bpk


def _in_maps(inputs: dict) -> list:
    x = np.asarray(inputs["x"], dtype=np.float32)
    assert x.shape == (B_FULL, NPIX), x.shape
    wp, bpk = _pack_weights(inputs)
    bf = ml_dtypes.bfloat16
    common = {"wp": wp, "bp": bpk}
    xr = x.reshape(N_CORES, B_CORE, NPIX)
    return [
        {"xt": np.ascontiguousarray(xr[c].T.astype(bf)), **common}
        for c in range(N_CORES)
    ]


def kernel(**inputs) -> np.ndarray:
    nc = _get_nc()
    res = run_bass_kernel_spmd(nc, _in_maps(inputs), list(range(N_CORES)))
    return np.concatenate(
        [res.results[c]["out"] for c in range(N_CORES)], axis=0
    )


if __name__ == "__main__":
    rng = np.random.default_rng(0)
    ins = {
        "x": rng.standard_normal((B_FULL, NPIX), dtype=np.float32),
        "conv_w": rng.standard_normal((3, 3), dtype=np.float32) * 0.1,
        "W1": rng.standard_normal((FLAT, HID), dtype=np.float32) * 0.04,
        "b1": np.zeros(HID, np.float32),
        "W2": rng.standard_normal((HID, HID), dtype=np.float32) * 0.06,
        "b2": np.zeros(HID, np.float32),
        "W3": rng.standard_normal((HID, NCLS), dtype=np.float32) * 0.06,
        "b3": np.zeros(NCLS, np.float32),
    }
    y = kernel(**ins)
    from numpy.lib.stride_tricks import sliding_window_view

    img = ins["x"].reshape(-1, IMG, IMG)
    win = sliding_window_view(img, (3, 3), axis=(1, 2))
    conv = np.einsum("bijkl,kl->bij", win, ins["conv_w"]).reshape(-1, FLAT)
    h = np.maximum(conv @ ins["W1"] + ins["b1"], 0)
    h = np.maximum(h @ ins["W2"] + ins["b2"], 0)
    ref = h @ ins["W3"] + ins["b3"]
    err = np.abs(y - ref).max() / (np.abs(ref).max() + 1e-9)
    print("max rel err vs numpy:", err)


# revision 7
# speedup vs baseline: 1.0672x; 1.0599x over previous
"""Trainium2 Bass kernel for DigitConvolutionalModel forward pass.

Model: x[B,784] -> 3x3 valid conv (single channel) -> flatten[676]
       -> relu(.@W1+b1) -> relu(.@W2+b2) -> .@W3+b3 -> [B,10]

Strategy (v3):
  - Pure data parallel: batch 32768 sharded 8 ways (4096 rows/core);
    weights replicated.
  - Conv folds into fc1 on the HOST (W1p = C @ W1 as 9 scatter-adds);
    x is transposed to pixel-major and cast to bf16 on the host. The
    device runs a pure bf16 matmul chain (fp32 PSUM accumulation);
    max rel err ~4e-3 vs the 2e-2 gate.
  - All replicated weights ship as ONE packed bf16 dram image (2 DMAs)
    plus one packed f32 bias image, issued after the first x tile so
    compute starts as early as possible.
  - The per-tile stages are software-pipelined 4 deep in emission
    order: iteration t runs fc1(t), fc2(t-1), fc3(t-2), store(t-3).
    Engines execute in order, so this gives every cross-engine
    dependency a full iteration (~6us) of slack and the PE streams
    back-to-back at its ~213ns/512-row bf16 issue rate.
  - Bias+ReLU PSUM evictions alternate between ScalarE and VectorE
    (GpSimd cannot read PSUM).
"""

import sys

for _p in (
    "/opt/trn_rl_repo",
    "/root/.axon_site",
    "/root/.axon_site/_ro/trn_rl_repo",
    "/root/.axon_site/_ro/pypackages",
):
    if _p not in sys.path:
        sys.path.append(_p)

from contextlib import ExitStack

import numpy as np
import ml_dtypes

import concourse.bass as bass
import concourse.tile as tile
from concourse import mybir
from concourse.bass_utils import run_bass_kernel_spmd
from concourse.masks import make_identity

F32 = mybir.dt.float32
BF16 = mybir.dt.bfloat16
AFT = mybir.ActivationFunctionType
ALU = mybir.AluOpType

B_FULL = 32768
N_CORES = 8
B_CORE = B_FULL // N_CORES  # 4096
IMG = 28
OHW = 26
FLAT = OHW * OHW  # 676
NPIX = IMG * IMG  # 784
HID = 300
NCLS = 10

BT = 512  # batch tile (matmul moving free dim)
NBT = B_CORE // BT  # 8
NBC = BT // 128  # 4

PC = 112  # pixel k-chunk width (784 = 7 x 112)
NPC = NPIX // PC  # 7
# 100+100+100 rather than 128+128+44: uniform chunks keep the PE's
# (row,col) tile size fixed at (128,128) across consecutive matmuls —
# mixed 64-row tiles from a 44-wide chunk cost ~95ns per reconfig.
H_CH = [(0, 100), (100, 100), (200, 100)]

# packed weight image columns (bf16): w1s | w2 chunks | w3 chunks
W1_COLS = NPC * HID  # 2100
W2_OFF = W1_COLS
W3_OFF = W2_OFF + 3 * HID  # 3000
WP_COLS = W3_OFF + 3 * NCLS  # 3030


def _legalize_single_wait(nc):
    """This walrus build accepts only one sync-wait per instruction; move
    extra waits onto NoOps inserted just before, on the same engine."""
    n = 0
    for fn in nc.m.functions:
        for bb in fn.blocks:
            new_insts = []
            for inst in bb.instructions:
                si = inst.sync_info
                if si is not None and si.on_wait and len(si.on_wait) > 1:
                    waits = list(si.on_wait)
                    for w in waits[:-1]:
                        nop = mybir.InstNoOp(
                            name=f"{inst.name}-w{n}",
                            sync_info=mybir.SyncInfo(on_wait=[w], on_update=[]),
                            bass_nofuse=True,
                            engine=inst.engine,
                        )
                        n += 1
                        nc.register_instruction(nop, overwrite=True)
                        new_insts.append(nop)
                    inst.sync_info = mybir.SyncInfo(
                        on_wait=[waits[-1]], on_update=list(si.on_update)
                    )
                new_insts.append(inst)
            bb.instructions = new_insts
    return n


def _emit(ctx: ExitStack, tc: tile.TileContext, xt, wp, bp, out):
    nc = tc.nc

    const = ctx.enter_context(tc.tile_pool(name="const", bufs=1))
    ps1p = ctx.enter_context(tc.tile_pool(name="ps1p", bufs=3, space="PSUM"))
    ps2p = ctx.enter_context(tc.tile_pool(name="ps2p", bufs=3, space="PSUM"))
    ps3p = ctx.enter_context(tc.tile_pool(name="ps3p", bufs=1, space="PSUM"))
    psop = ctx.enter_context(tc.tile_pool(name="psop", bufs=1, space="PSUM"))
    xtp = ctx.enter_context(tc.tile_pool(name="xtp", bufs=4))
    hp_ = ctx.enter_context(tc.tile_pool(name="hp", bufs=2))
    op_ = ctx.enter_context(tc.tile_pool(name="op", bufs=2))
    obp = ctx.enter_context(tc.tile_pool(name="obp", bufs=4))

    def load_x(t):
        """One 3D DMA: xt dram [784, 4096] cols [t*512,(t+1)*512) -> SBUF
        [112, 7, 512] bf16 (pixel chunk-major)."""
        xtile = xtp.tile([PC, NPC * BT], BF16, name="xt", tag="xt")
        nc.sync.dma_start(
            xtile[:, :].rearrange("p (c n) -> p c n", c=NPC),
            xt[:, t * BT : (t + 1) * BT].rearrange("(c p) n -> p c n", p=PC),
        )
        return xtile

    # x tile 0 first (fc1(0) blocks on it), then weights, then more x.
    xts = {0: load_x(0)}

    wsb = const.tile([128, WP_COLS], BF16, name="wsb")
    nc.sync.dma_start(wsb[:, 0:W1_COLS], wp[:, 0:W1_COLS])
    bsb = const.tile([128, 7], F32, name="bsb")
    nc.sync.dma_start(bsb[:, :], bp[:, :])
    nc.sync.dma_start(wsb[:, W1_COLS:WP_COLS], wp[:, W1_COLS:WP_COLS])

    for t in (1, 2):
        xts[t] = load_x(t)

    ident = const.tile([128, 128], F32, name="ident")
    make_identity(nc, ident)

    def w1_ap(c, h0, hp):
        return wsb[0:PC, c * HID + h0 : c * HID + h0 + hp]

    def w2_ap(hc, hp, g0, gp):
        return wsb[0:hp, W2_OFF + hc * HID + g0 : W2_OFF + hc * HID + g0 + gp]

    def w3_ap(hc, hp):
        return wsb[0:hp, W3_OFF + hc * NCLS : W3_OFF + (hc + 1) * NCLS]

    def bias_ap(col, hp):
        return bsb[0:hp, col : col + 1]

    h1s = {}
    h2s = {}
    obs = {}

    def fc1(t):
        xtile = xts.pop(t)
        h1 = []
        for hc, (h0, hp) in enumerate(H_CH):
            ps = ps1p.tile([128, BT], F32, name="ps1", tag="ps1")
            for c in range(NPC):
                nc.tensor.matmul(
                    ps[0:hp, 0:BT],
                    w1_ap(c, h0, hp),
                    xtile[:, c * BT : (c + 1) * BT],
                    start=(c == 0),
                    stop=(c == NPC - 1),
                )
            h = hp_.tile([hp, BT], BF16, name=f"h1_{hc}", tag=f"h1_{hc}")
            if hc in (0, 2):
                nc.scalar.activation(
                    h[:, :], ps[0:hp, 0:BT], AFT.Relu, bias=bias_ap(hc, hp)
                )
            else:
                nc.vector.tensor_scalar(
                    h[:, :], ps[0:hp, 0:BT], bias_ap(hc, hp), 0.0,
                    ALU.add, ALU.max,
                )
            h1.append(h)
        h1s[t] = h1
        if t + 3 < NBT:
            xts[t + 3] = load_x(t + 3)

    def fc2(t):
        # g-outer / k-inner: consecutive matmuls share a PSUM bank (bank
        # switches between back-to-back matmuls cost ~95ns on HW). All h1
        # chunks are ready a full iteration ahead, so k-inner never stalls.
        h1 = h1s.pop(t)
        h2 = []
        for g, (g0, gp) in enumerate(H_CH):
            ps = ps2p.tile([128, BT], F32, name=f"ps2_{g}", tag="ps2")
            for hc, (h0, hp) in enumerate(H_CH):
                nc.tensor.matmul(
                    ps[0:gp, 0:BT],
                    w2_ap(hc, hp, g0, gp),
                    h1[hc][:, :],
                    start=(hc == 0),
                    stop=(hc == len(H_CH) - 1),
                )
            h = hp_.tile([gp, BT], BF16, name=f"h2_{g}", tag=f"h2_{g}")
            if g in (0, 2):
                nc.vector.tensor_scalar(
                    h[:, :], ps[0:gp, 0:BT], bias_ap(3 + g, gp), 0.0,
                    ALU.add, ALU.max,
                )
            else:
                nc.scalar.activation(
                    h[:, :], ps[0:gp, 0:BT], AFT.Relu, bias=bias_ap(3 + g, gp)
                )
            h2.append(h)
        h2s[t] = h2

    def fc3(t):
        h2 = h2s.pop(t)
        ps3 = ps3p.tile([NCLS, BT], F32, name="ps3", tag="ps3")
        for hc, (h0, hp) in enumerate(H_CH):
            nc.tensor.matmul(
                ps3[0:NCLS, 0:BT],
                w3_ap(hc, hp),
                h2[hc][:, :],
                start=(hc == 0),
                stop=(hc == len(H_CH) - 1),
            )
        ob = op_.tile([NCLS, BT], F32, name="ob", tag="ob")
        nc.scalar.activation(
            ob[:, :], ps3[0:NCLS, 0:BT], AFT.Identity, bias=bias_ap(6, NCLS)
        )
        obs[t] = ob

    def store(t):
        ob = obs.pop(t)
        r0 = t * BT
        po = psop.tile([128, NBC * NCLS], F32, name="po", tag="po")
        for bc in range(NBC):
            nc.tensor.transpose(
                po[0:128, bc * NCLS : (bc + 1) * NCLS],
                ob[:, bc * 128 : (bc + 1) * 128],
                ident[0:NCLS, 0:NCLS],
            )
        os_ = obp.tile([128, NBC * NCLS], F32, name="os", tag="os")
        nc.vector.tensor_copy(os_[:, :], po[0:128, 0 : NBC * NCLS])
        nc.sync.dma_start(
            out[r0 : r0 + BT, :].rearrange("(bc b) c -> b bc c", bc=NBC),
            os_[:, :].rearrange("b (bc c) -> b bc c", bc=NBC),
        )

    for it in range(NBT + 3):
        if it < NBT:
            fc1(it)
        if 0 <= it - 1 < NBT:
            fc2(it - 1)
        if 0 <= it - 2 < NBT:
            fc3(it - 2)
        if 0 <= it - 3 < NBT:
            store(it - 3)


def _build_w1p(conv_w: np.ndarray, W1: np.ndarray) -> np.ndarray:
    """Fold the 3x3 valid conv into fc1: W1p[p, :] = sum over taps landing
    on pixel p of conv_w[dy,dx] * W1[q(p,dy,dx), :]. Zero-FLOP scatter-add."""
    w1p = np.zeros((NPIX, HID), np.float32)
    oi = np.arange(OHW)
    oj = np.arange(OHW)
    q = (oi[:, None] * OHW + oj[None, :]).ravel()
    for dy in range(3):
        for dx in range(3):
            p = ((oi[:, None] + dy) * IMG + (oj[None, :] + dx)).ravel()
            np.add.at(w1p, p, conv_w[dy, dx] * W1[q, :])
    return w1p


_NC_CACHE: list = []


def _get_nc():
    if _NC_CACHE:
        return _NC_CACHE[0]
    nc = bass.Bass("TRN2", target_bir_lowering=False, debug=False)
    xt = nc.dram_tensor("xt", [NPIX, B_CORE], BF16, kind="ExternalInput").ap()
    wp = nc.dram_tensor("wp", [128, WP_COLS], BF16, kind="ExternalInput").ap()
    bp = nc.dram_tensor("bp", [128, 7], F32, kind="ExternalInput").ap()
    out = nc.dram_tensor("out", [B_CORE, NCLS], F32, kind="ExternalOutput").ap()
    with tile.TileContext(nc) as tc:
        with ExitStack() as ctx:
            _emit(ctx, tc, xt, wp, bp, out)
    _legalize_single_wait(nc)
    _NC_CACHE.append(nc)
    return nc


def _pack_weights(inputs: dict) -> tuple:
    bf = ml_dtypes.bfloat16
    w1p = _build_w1p(
        np.asarray(inputs["conv_w"], dtype=np.float32),
        np.asarray(inputs["W1"], dtype=np.float32),
    )
    w2 = np.asarray(inputs["W2"], np.float32)
    w3 = np.asarray(inputs["W3"], np.float32)
    wp = np.zeros((128, WP_COLS), bf)
    # w1s: [112, 7, 300] pixel chunk-major
    wp[0:PC, 0:W1_COLS] = (
        w1p.reshape(NPC, PC, HID).transpose(1, 0, 2).reshape(PC, W1_COLS).astype(bf)
    )
    for hc, (h0, hp) in enumerate(H_CH):
        wp[0:hp, W2_OFF + hc * HID : W2_OFF + (hc + 1) * HID] = w2[
            h0 : h0 + hp, :
        ].astype(bf)
        wp[0:hp, W3_OFF + hc * NCLS : W3_OFF + (hc + 1) * NCLS] = w3[
            h0 : h0 + hp, :
        ].astype(bf)
    bpk = np.zeros((128, 7), np.float32)
    b1 = np.asarray(inputs["b1"], np.float32)
    b2 = np.asarray(inputs["b2"], np.float32)
    b3 = np.asarray(inputs["b3"], np.float32)
    for hc, (h0, hp) in enumerate(H_CH):
        bpk[0:hp, hc] = b1[h0 : h0 + hp]
        bpk[0:hp, 3 + hc] = b2[h0 : h0 + hp]
    bpk[0:NCLS, 6] = b3
    return wp, bpk


def _in_maps(inputs: dict) -> list:
    x = np.asarray(inputs["x"], dtype=np.float32)
    assert x.shape == (B_FULL, NPIX), x.shape
    wp, bpk = _pack_weights(inputs)
    bf = ml_dtypes.bfloat16
    common = {"wp": wp, "bp": bpk}
    xr = x.reshape(N_CORES, B_CORE, NPIX)
    return [
        {"xt": np.ascontiguousarray(xr[c].T.astype(bf)), **common}
        for c in range(N_CORES)
    ]


def kernel(**inputs) -> np.ndarray:
    nc = _get_nc()
    res = run_bass_kernel_spmd(nc, _in_maps(inputs), list(range(N_CORES)))
    return np.concatenate(
        [res.results[c]["out"] for c in range(N_CORES)], axis=0
    )


if __name__ == "__main__":
    rng = np.random.default_rng(0)
    ins = {
        "x": rng.standard_normal((B_FULL, NPIX), dtype=np.float32),
        "conv_w": rng.standard_normal((3, 3), dtype=np.float32) * 0.1,
        "W1": rng.standard_normal((FLAT, HID), dtype=np.float32) * 0.04,
        "b1": np.zeros(HID, np.float32),
        "W2": rng.standard_normal((HID, HID), dtype=np.float32) * 0.06,
        "b2": np.zeros(HID, np.float32),
        "W3": rng.standard_normal((HID, NCLS), dtype=np.float32) * 0.06,
        "b3": np.zeros(NCLS, np.float32),
    }
    y = kernel(**ins)
    from numpy.lib.stride_tricks import sliding_window_view

    img = ins["x"].reshape(-1, IMG, IMG)
    win = sliding_window_view(img, (3, 3), axis=(1, 2))
    conv = np.einsum("bijkl,kl->bij", win, ins["conv_w"]).reshape(-1, FLAT)
    h = np.maximum(conv @ ins["W1"] + ins["b1"], 0)
    h = np.maximum(h @ ins["W2"] + ins["b2"], 0)
    ref = h @ ins["W3"] + ins["b3"]
    err = np.abs(y - ref).max() / (np.abs(ref).max() + 1e-9)
    print("max rel err vs numpy:", err)


# revision 17
# speedup vs baseline: 1.0968x; 1.0278x over previous
"""Trainium2 Bass kernel for DigitConvolutionalModel forward pass.

Model: x[B,784] -> 3x3 valid conv (single channel) -> flatten[676]
       -> relu(.@W1+b1) -> relu(.@W2+b2) -> .@W3+b3 -> [B,10]

Strategy (v3):
  - Pure data parallel: batch 32768 sharded 8 ways (4096 rows/core);
    weights replicated.
  - Conv folds into fc1 on the HOST (W1p = C @ W1 as 9 scatter-adds);
    x is transposed to pixel-major and cast to bf16 on the host. The
    device runs a pure bf16 matmul chain (fp32 PSUM accumulation);
    max rel err ~4e-3 vs the 2e-2 gate.
  - All replicated weights ship as ONE packed bf16 dram image (2 DMAs)
    plus one packed f32 bias image, issued after the first x tile so
    compute starts as early as possible.
  - The per-tile stages are software-pipelined 4 deep in emission
    order: iteration t runs fc1(t), fc2(t-1), fc3(t-2), store(t-3).
    Engines execute in order, so this gives every cross-engine
    dependency a full iteration (~6us) of slack and the PE streams
    back-to-back at its ~213ns/512-row bf16 issue rate.
  - Bias+ReLU PSUM evictions alternate between ScalarE and VectorE
    (GpSimd cannot read PSUM).
"""

import sys

for _p in (
    "/opt/trn_rl_repo",
    "/root/.axon_site",
    "/root/.axon_site/_ro/trn_rl_repo",
    "/root/.axon_site/_ro/pypackages",
):
    if _p not in sys.path:
        sys.path.append(_p)

from contextlib import ExitStack

import numpy as np
import ml_dtypes

import concourse.bass as bass
import concourse.tile as tile
from concourse import mybir
from concourse.bass_utils import run_bass_kernel_spmd

F32 = mybir.dt.float32
BF16 = mybir.dt.bfloat16
AFT = mybir.ActivationFunctionType
ALU = mybir.AluOpType

B_FULL = 32768
N_CORES = 8
B_CORE = B_FULL // N_CORES  # 4096
IMG = 28
OHW = 26
FLAT = OHW * OHW  # 676
NPIX = IMG * IMG  # 784
HID = 300
NCLS = 10

BT = 512  # batch tile (matmul moving free dim)
NBT = B_CORE // BT  # 8
NBC = BT // 128  # 4

PC = 112  # pixel k-chunk width (784 = 7 x 112)
NPC = NPIX // PC  # 7
# 100+100+100 rather than 128+128+44: uniform chunks keep the PE's
# (row,col) tile size fixed at (128,128) across consecutive matmuls —
# mixed 64-row tiles from a 44-wide chunk cost ~95ns per reconfig.
H_CH = [(0, 100), (100, 100), (200, 100)]

# packed weight image columns (bf16): w1s | w2 chunks | w3 chunks.
# w3 is padded 10 -> 128 output columns so fc3's matmuls keep the PE
# (row,col) tile at (128,128) — a (128,32) tile reconfig costs ~95ns.
NC_PAD = 128
W1_COLS = NPC * HID  # 2100
W2_OFF = W1_COLS
W3_OFF = W2_OFF + 3 * HID  # 3000
WP_COLS = W3_OFF + 3 * NC_PAD  # 3384


def _legalize_single_wait(nc):
    """This walrus build accepts only one sync-wait per instruction; move
    extra waits onto NoOps inserted just before, on the same engine."""
    n = 0
    for fn in nc.m.functions:
        for bb in fn.blocks:
            new_insts = []
            for inst in bb.instructions:
                si = inst.sync_info
                if si is not None and si.on_wait and len(si.on_wait) > 1:
                    waits = list(si.on_wait)
                    for w in waits[:-1]:
                        nop = mybir.InstNoOp(
                            name=f"{inst.name}-w{n}",
                            sync_info=mybir.SyncInfo(on_wait=[w], on_update=[]),
                            bass_nofuse=True,
                            engine=inst.engine,
                        )
                        n += 1
                        nc.register_instruction(nop, overwrite=True)
                        new_insts.append(nop)
                    inst.sync_info = mybir.SyncInfo(
                        on_wait=[waits[-1]], on_update=list(si.on_update)
                    )
                new_insts.append(inst)
            bb.instructions = new_insts
    return n


def _emit(ctx: ExitStack, tc: tile.TileContext, xt, wp, bp, out):
    nc = tc.nc

    const = ctx.enter_context(tc.tile_pool(name="const", bufs=1))
    ps1p = ctx.enter_context(tc.tile_pool(name="ps1p", bufs=3, space="PSUM"))
    ps2p = ctx.enter_context(tc.tile_pool(name="ps2p", bufs=3, space="PSUM"))
    ps3p = ctx.enter_context(tc.tile_pool(name="ps3p", bufs=2, space="PSUM"))
    xtp = ctx.enter_context(tc.tile_pool(name="xtp", bufs=4))
    hp_ = ctx.enter_context(tc.tile_pool(name="hp", bufs=2))
    op_ = ctx.enter_context(tc.tile_pool(name="op", bufs=2))

    def load_x(t):
        """One 3D DMA: xt dram [784, 4096] cols [t*512,(t+1)*512) -> SBUF
        [112, 7, 512] bf16 (pixel chunk-major)."""
        xtile = xtp.tile([PC, NPC * BT], BF16, name="xt", tag="xt")
        nc.sync.dma_start(
            xtile[:, :].rearrange("p (c n) -> p c n", c=NPC),
            xt[:, t * BT : (t + 1) * BT].rearrange("(c p) n -> p c n", p=PC),
        )
        return xtile

    # x tile 0 first (fc1(0) blocks on it), then weights, then more x.
    xts = {0: load_x(0)}

    wsb = const.tile([128, WP_COLS], BF16, name="wsb")
    nc.sync.dma_start(wsb[:, 0:W1_COLS], wp[:, 0:W1_COLS])
    bsb = const.tile([128, 7], F32, name="bsb")
    nc.sync.dma_start(bsb[:, :], bp[:, :])
    nc.sync.dma_start(wsb[:, W1_COLS:WP_COLS], wp[:, W1_COLS:WP_COLS])

    for t in (1, 2):
        xts[t] = load_x(t)

    def w1_ap(c, h0, hp):
        return wsb[0:PC, c * HID + h0 : c * HID + h0 + hp]

    def w2_ap(hc, hp, g0, gp):
        return wsb[0:hp, W2_OFF + hc * HID + g0 : W2_OFF + hc * HID + g0 + gp]

    def w3_ap(hc, hp):
        return wsb[0:hp, W3_OFF + hc * NC_PAD : W3_OFF + hc * NC_PAD + NC_PAD]

    def bias_ap(col, hp):
        return bsb[0:hp, col : col + 1]

    h1s = {}
    h2s = {}

    def fc1(t):
        xtile = xts.pop(t)
        h1 = []
        for hc, (h0, hp) in enumerate(H_CH):
            ps = ps1p.tile([128, BT], F32, name="ps1", tag="ps1")
            for c in range(NPC):
                nc.tensor.matmul(
                    ps[0:hp, 0:BT],
                    w1_ap(c, h0, hp),
                    xtile[:, c * BT : (c + 1) * BT],
                    start=(c == 0),
                    stop=(c == NPC - 1),
                )
            h = hp_.tile([hp, BT], BF16, name=f"h1_{hc}", tag=f"h1_{hc}")
            if hc in (0, 2):
                nc.scalar.activation(
                    h[:, :], ps[0:hp, 0:BT], AFT.Relu, bias=bias_ap(hc, hp)
                )
            else:
                nc.vector.tensor_scalar(
                    h[:, :], ps[0:hp, 0:BT], bias_ap(hc, hp), 0.0,
                    ALU.add, ALU.max,
                )
            h1.append(h)
        h1s[t] = h1
        if t + 3 < NBT:
            xts[t + 3] = load_x(t + 3)

    def fc2(t):
        # g-outer / k-inner: consecutive matmuls share a PSUM bank (bank
        # switches between back-to-back matmuls cost ~95ns on HW). All h1
        # chunks are ready a full iteration ahead, so k-inner never stalls.
        h1 = h1s.pop(t)
        h2 = []
        for g, (g0, gp) in enumerate(H_CH):
            ps = ps2p.tile([128, BT], F32, name=f"ps2_{g}", tag="ps2")
            for hc, (h0, hp) in enumerate(H_CH):
                nc.tensor.matmul(
                    ps[0:gp, 0:BT],
                    w2_ap(hc, hp, g0, gp),
                    h1[hc][:, :],
                    start=(hc == 0),
                    stop=(hc == len(H_CH) - 1),
                )
            h = hp_.tile([gp, BT], BF16, name=f"h2_{g}", tag=f"h2_{g}")
            if g in (0, 2):
                nc.vector.tensor_scalar(
                    h[:, :], ps[0:gp, 0:BT], bias_ap(3 + g, gp), 0.0,
                    ALU.add, ALU.max,
                )
            else:
                nc.scalar.activation(
                    h[:, :], ps[0:gp, 0:BT], AFT.Relu, bias=bias_ap(3 + g, gp)
                )
            h2.append(h)
        h2s[t] = h2

    def fc3(t):
        # output stays hidden-major [10, BT]; the host transposes. Padded
        # 128-wide stationary keeps the PE tile at (128,128).
        h2 = h2s.pop(t)
        ps3 = ps3p.tile([128, BT], F32, name="ps3", tag="ps3")
        for hc, (h0, hp) in enumerate(H_CH):
            nc.tensor.matmul(
                ps3[:, 0:BT],
                w3_ap(hc, hp),
                h2[hc][:, :],
                start=(hc == 0),
                stop=(hc == len(H_CH) - 1),
            )
        ob = op_.tile([NCLS, BT], F32, name="ob", tag="ob")
        nc.scalar.activation(
            ob[:, :], ps3[0:NCLS, 0:BT], AFT.Identity, bias=bias_ap(6, NCLS)
        )
        nc.sync.dma_start(out[:, t * BT : (t + 1) * BT], ob[:, :])

    for it in range(NBT + 2):
        if it < NBT:
            fc1(it)
        if 0 <= it - 1 < NBT:
            fc2(it - 1)
        if 0 <= it - 2 < NBT:
            fc3(it - 2)


def _build_w1p(conv_w: np.ndarray, W1: np.ndarray) -> np.ndarray:
    """Fold the 3x3 valid conv into fc1: W1p[p, :] = sum over taps landing
    on pixel p of conv_w[dy,dx] * W1[q(p,dy,dx), :]. Zero-FLOP scatter-add."""
    w1p = np.zeros((NPIX, HID), np.float32)
    oi = np.arange(OHW)
    oj = np.arange(OHW)
    q = (oi[:, None] * OHW + oj[None, :]).ravel()
    for dy in range(3):
        for dx in range(3):
            p = ((oi[:, None] + dy) * IMG + (oj[None, :] + dx)).ravel()
            np.add.at(w1p, p, conv_w[dy, dx] * W1[q, :])
    return w1p


_NC_CACHE: list = []


def _get_nc():
    if _NC_CACHE:
        return _NC_CACHE[0]
    nc = bass.Bass("TRN2", target_bir_lowering=False, debug=False)
    xt = nc.dram_tensor("xt", [NPIX, B_CORE], BF16, kind="ExternalInput").ap()
    wp = nc.dram_tensor("wp", [128, WP_COLS], BF16, kind="ExternalInput").ap()
    bp = nc.dram_tensor("bp", [128, 7], F32, kind="ExternalInput").ap()
    out = nc.dram_tensor("out", [NCLS, B_CORE], F32, kind="ExternalOutput").ap()
    with tile.TileContext(nc) as tc:
        with ExitStack() as ctx:
            _emit(ctx, tc, xt, wp, bp, out)
    _legalize_single_wait(nc)
    _NC_CACHE.append(nc)
    return nc


def _pack_weights(inputs: dict) -> tuple:
    bf = ml_dtypes.bfloat16
    w1p = _build_w1p(
        np.asarray(inputs["conv_w"], dtype=np.float32),
        np.asarray(inputs["W1"], dtype=np.float32),
    )
    w2 = np.asarray(inputs["W2"], np.float32)
    w3 = np.asarray(inputs["W3"], np.float32)
    wp = np.zeros((128, WP_COLS), bf)
    # w1s: [112, 7, 300] pixel chunk-major
    wp[0:PC, 0:W1_COLS] = (
        w1p.reshape(NPC, PC, HID).transpose(1, 0, 2).reshape(PC, W1_COLS).astype(bf)
    )
    for hc, (h0, hp) in enumerate(H_CH):
        wp[0:hp, W2_OFF + hc * HID : W2_OFF + (hc + 1) * HID] = w2[
            h0 : h0 + hp, :
        ].astype(bf)
        wp[0:hp, W3_OFF + hc * NC_PAD : W3_OFF + hc * NC_PAD + NCLS] = w3[
            h0 : h0 + hp, :
        ].astype(bf)
    bpk = np.zeros((128, 7), np.float32)
    b1 = np.asarray(inputs["b1"], np.float32)
    b2 = np.asarray(inputs["b2"], np.float32)
    b3 = np.asarray(inputs["b3"], np.float32)
    for hc, (h0, hp) in enumerate(H_CH):
        bpk[0:hp, hc] = b1[h0 : h0 + hp]
        bpk[0:hp, 3 + hc] = b2[h0 : h0 + hp]
    bpk[0:NCLS, 6] = b3
    return wp, bpk


def _in_maps(inputs: dict) -> list:
    x = np.asarray(inputs["x"], dtype=np.float32)
    assert x.shape == (B_FULL, NPIX), x.shape
    wp, bpk = _pack_weights(inputs)
    bf = ml_dtypes.bfloat16
    common = {"wp": wp, "bp": bpk}
    xr = x.reshape(N_CORES, B_CORE, NPIX)
    return [
        {"xt": np.ascontiguousarray(xr[c].T.astype(bf)), **common}
        for c in range(N_CORES)
    ]


def kernel(**inputs) -> np.ndarray:
    nc = _get_nc()
    res = run_bass_kernel_spmd(nc, _in_maps(inputs), list(range(N_CORES)))
    return np.concatenate(
        [np.ascontiguousarray(res.results[c]["out"].T) for c in range(N_CORES)],
        axis=0,
    )


if __name__ == "__main__":
    rng = np.random.default_rng(0)
    ins = {
        "x": rng.standard_normal((B_FULL, NPIX), dtype=np.float32),
        "conv_w": rng.standard_normal((3, 3), dtype=np.float32) * 0.1,
        "W1": rng.standard_normal((FLAT, HID), dtype=np.float32) * 0.04,
        "b1": np.zeros(HID, np.float32),
        "W2": rng.standard_normal((HID, HID), dtype=np.float32) * 0.06,
        "b2": np.zeros(HID, np.float32),
        "W3": rng.standard_normal((HID, NCLS), dtype=np.float32) * 0.06,
        "b3": np.zeros(NCLS, np.float32),
    }
    y = kernel(**ins)
    from numpy.lib.stride_tricks import sliding_window_view

    img = ins["x"].reshape(-1, IMG, IMG)
    win = sliding_window_view(img, (3, 3), axis=(1, 2))
    conv = np.einsum("bijkl,kl->bij", win, ins["conv_w"]).reshape(-1, FLAT)
    h = np.maximum(conv @ ins["W1"] + ins["b1"], 0)
    h = np.maximum(h @ ins["W2"] + ins["b2"], 0)
    ref = h @ ins["W3"] + ins["b3"]
    err = np.abs(y - ref).max() / (np.abs(ref).max() + 1e-9)
    print("max rel err vs numpy:", err)


# revision 21
# speedup vs baseline: 1.1433x; 1.0424x over previous
"""Trainium2 Bass kernel for DigitConvolutionalModel forward pass.

Model: x[B,784] -> 3x3 valid conv (single channel) -> flatten[676]
       -> relu(.@W1+b1) -> relu(.@W2+b2) -> .@W3+b3 -> [B,10]

Strategy (v3):
  - Pure data parallel: batch 32768 sharded 8 ways (4096 rows/core);
    weights replicated.
  - Conv folds into fc1 on the HOST (W1p = C @ W1 as 9 scatter-adds);
    x is transposed to pixel-major and cast to bf16 on the host. The
    device runs a pure bf16 matmul chain (fp32 PSUM accumulation);
    max rel err ~4e-3 vs the 2e-2 gate.
  - All replicated weights ship as ONE packed bf16 dram image (2 DMAs)
    plus one packed f32 bias image, issued after the first x tile so
    compute starts as early as possible.
  - The per-tile stages are software-pipelined 4 deep in emission
    order: iteration t runs fc1(t), fc2(t-1), fc3(t-2), store(t-3).
    Engines execute in order, so this gives every cross-engine
    dependency a full iteration (~6us) of slack and the PE streams
    back-to-back at its ~213ns/512-row bf16 issue rate.
  - Bias+ReLU PSUM evictions alternate between ScalarE and VectorE
    (GpSimd cannot read PSUM).
"""

import sys

for _p in (
    "/opt/trn_rl_repo",
    "/root/.axon_site",
    "/root/.axon_site/_ro/trn_rl_repo",
    "/root/.axon_site/_ro/pypackages",
):
    if _p not in sys.path:
        sys.path.append(_p)

from contextlib import ExitStack

import numpy as np
import ml_dtypes

import concourse.bass as bass
import concourse.tile as tile
from concourse import mybir
from concourse.bass_utils import run_bass_kernel_spmd

F32 = mybir.dt.float32
BF16 = mybir.dt.bfloat16
AFT = mybir.ActivationFunctionType
ALU = mybir.AluOpType

B_FULL = 32768
N_CORES = 8
B_CORE = B_FULL // N_CORES  # 4096
IMG = 28
OHW = 26
FLAT = OHW * OHW  # 676
NPIX = IMG * IMG  # 784
HID = 300
NCLS = 10

BT = 512  # batch tile (matmul moving free dim)
NBT = B_CORE // BT  # 8
NBC = BT // 128  # 4

PC = 112  # pixel k-chunk width (784 = 7 x 112)
NPC = NPIX // PC  # 7
# 100+100+100 rather than 128+128+44: uniform chunks keep the PE's
# (row,col) tile size fixed at (128,128) across consecutive matmuls —
# mixed 64-row tiles from a 44-wide chunk cost ~95ns per reconfig.
H_CH = [(0, 100), (100, 100), (200, 100)]

# packed weight image columns (bf16): w1s | w2 chunks | w3 chunks.
# w3 is padded 10 -> 128 output columns so fc3's matmuls keep the PE
# (row,col) tile at (128,128) — a (128,32) tile reconfig costs ~95ns.
NC_PAD = 128
W1_COLS = NPC * HID  # 2100
W2_OFF = W1_COLS
W3_OFF = W2_OFF + 3 * HID  # 3000
WP_COLS = W3_OFF + 3 * NC_PAD  # 3384


def _legalize_single_wait(nc):
    """This walrus build accepts only one sync-wait per instruction; move
    extra waits onto NoOps inserted just before, on the same engine."""
    n = 0
    for fn in nc.m.functions:
        for bb in fn.blocks:
            new_insts = []
            for inst in bb.instructions:
                si = inst.sync_info
                if si is not None and si.on_wait and len(si.on_wait) > 1:
                    waits = list(si.on_wait)
                    for w in waits[:-1]:
                        nop = mybir.InstNoOp(
                            name=f"{inst.name}-w{n}",
                            sync_info=mybir.SyncInfo(on_wait=[w], on_update=[]),
                            bass_nofuse=True,
                            engine=inst.engine,
                        )
                        n += 1
                        nc.register_instruction(nop, overwrite=True)
                        new_insts.append(nop)
                    inst.sync_info = mybir.SyncInfo(
                        on_wait=[waits[-1]], on_update=list(si.on_update)
                    )
                new_insts.append(inst)
            bb.instructions = new_insts
    return n


def _emit(ctx: ExitStack, tc: tile.TileContext, xt, wp, bp, out):
    nc = tc.nc

    const = ctx.enter_context(tc.tile_pool(name="const", bufs=1))
    ps1p = ctx.enter_context(tc.tile_pool(name="ps1p", bufs=3, space="PSUM"))
    ps2p = ctx.enter_context(tc.tile_pool(name="ps2p", bufs=3, space="PSUM"))
    ps3p = ctx.enter_context(tc.tile_pool(name="ps3p", bufs=2, space="PSUM"))
    xtp = ctx.enter_context(tc.tile_pool(name="xtp", bufs=4))
    hp_ = ctx.enter_context(tc.tile_pool(name="hp", bufs=2))
    op_ = ctx.enter_context(tc.tile_pool(name="op", bufs=2))

    def load_x(t):
        """One 3D DMA: xt dram [784, 4096] cols [t*512,(t+1)*512) -> SBUF
        [112, 7, 512] bf16 (pixel chunk-major)."""
        xtile = xtp.tile([PC, NPC * BT], BF16, name="xt", tag="xt")
        nc.sync.dma_start(
            xtile[:, :].rearrange("p (c n) -> p c n", c=NPC),
            xt[:, t * BT : (t + 1) * BT].rearrange("(c p) n -> p c n", p=PC),
        )
        return xtile

    # Only what fc1(0) needs ships before compute: x tile 0, the fc1
    # weights, and the biases. Everything else (w2/w3, x tiles 1+) is
    # issued from inside fc1(0) so its DMA packets don't steal bandwidth
    # from the critical first transfers.
    xts = {0: load_x(0)}

    w1s = const.tile([PC, W1_COLS], BF16, name="w1s")
    nc.sync.dma_start(w1s[:, :], wp[0:PC, 0:W1_COLS])
    bsb = const.tile([128, 7], F32, name="bsb")
    nc.sync.dma_start(bsb[:, :], bp[:, :])
    wsb2 = const.tile([128, WP_COLS - W1_COLS], BF16, name="wsb2")

    # Dummy matmuls on an uninitialized SBUF tile while the first DMAs
    # fly: the PE p-state ramps to 2.4 GHz only after ~3us of continuous
    # busy, so burn the DMA wait instead of the first real batch tile.
    warm = const.tile([128, BT], BF16, name="warm")
    nc.gpsimd.memset(warm[:, :], 0)
    wps = ps1p.tile([128, BT], F32, name="warmps", tag="ps1")
    for _ in range(10):
        nc.tensor.matmul(
            wps[0:100, 0:BT], warm[:, 0:100], warm[:, :], start=True, stop=True,
            skip_group_check=True,
        )

    def w1_ap(c, h0, hp):
        return w1s[0:PC, c * HID + h0 : c * HID + h0 + hp]

    def w2_ap(hc, hp, g0, gp):
        return wsb2[0:hp, hc * HID + g0 : hc * HID + g0 + gp]

    def w3_ap(hc, hp):
        w3o = 3 * HID
        return wsb2[0:hp, w3o + hc * NC_PAD : w3o + hc * NC_PAD + NC_PAD]

    def bias_ap(col, hp):
        return bsb[0:hp, col : col + 1]

    h1s = {}
    h2s = {}

    # DMA work deferred into fc1(0)'s emission window, drained one item
    # per m-chunk so the sync engine interleaves issue with compute.
    deferred = [lambda: nc.sync.dma_start(wsb2[:, :], wp[:, W1_COLS:WP_COLS])]
    for t in (1, 2):
        deferred.append(lambda t=t: xts.__setitem__(t, load_x(t)))

    def fc1(t):
        xtile = xts.pop(t)
        h1 = []
        for hc, (h0, hp) in enumerate(H_CH):
            ps = ps1p.tile([128, BT], F32, name="ps1", tag="ps1")
            for c in range(NPC):
                nc.tensor.matmul(
                    ps[0:hp, 0:BT],
                    w1_ap(c, h0, hp),
                    xtile[:, c * BT : (c + 1) * BT],
                    start=(c == 0),
                    stop=(c == NPC - 1),
                )
            h = hp_.tile([hp, BT], BF16, name=f"h1_{hc}", tag=f"h1_{hc}")
            if hc in (0, 2):
                nc.scalar.activation(
                    h[:, :], ps[0:hp, 0:BT], AFT.Relu, bias=bias_ap(hc, hp)
                )
            else:
                nc.vector.tensor_scalar(
                    h[:, :], ps[0:hp, 0:BT], bias_ap(hc, hp), 0.0,
                    ALU.add, ALU.max,
                )
            h1.append(h)
            if deferred:
                deferred.pop(0)()
        h1s[t] = h1
        if t + 3 < NBT:
            xts[t + 3] = load_x(t + 3)

    def fc2(t):
        # g-outer / k-inner: consecutive matmuls share a PSUM bank (bank
        # switches between back-to-back matmuls cost ~95ns on HW). All h1
        # chunks are ready a full iteration ahead, so k-inner never stalls.
        h1 = h1s.pop(t)
        h2 = []
        for g, (g0, gp) in enumerate(H_CH):
            ps = ps2p.tile([128, BT], F32, name=f"ps2_{g}", tag="ps2")
            for hc, (h0, hp) in enumerate(H_CH):
                nc.tensor.matmul(
                    ps[0:gp, 0:BT],
                    w2_ap(hc, hp, g0, gp),
                    h1[hc][:, :],
                    start=(hc == 0),
                    stop=(hc == len(H_CH) - 1),
                )
            h = hp_.tile([gp, BT], BF16, name=f"h2_{g}", tag=f"h2_{g}")
            if g in (0, 2):
                nc.vector.tensor_scalar(
                    h[:, :], ps[0:gp, 0:BT], bias_ap(3 + g, gp), 0.0,
                    ALU.add, ALU.max,
                )
            else:
                nc.scalar.activation(
                    h[:, :], ps[0:gp, 0:BT], AFT.Relu, bias=bias_ap(3 + g, gp)
                )
            h2.append(h)
        h2s[t] = h2

    def fc3(t):
        # output stays hidden-major [10, BT]; the host transposes. Padded
        # 128-wide stationary keeps the PE tile at (128,128).
        h2 = h2s.pop(t)
        ps3 = ps3p.tile([128, BT], F32, name="ps3", tag="ps3")
        for hc, (h0, hp) in enumerate(H_CH):
            nc.tensor.matmul(
                ps3[:, 0:BT],
                w3_ap(hc, hp),
                h2[hc][:, :],
                start=(hc == 0),
                stop=(hc == len(H_CH) - 1),
            )
        ob = op_.tile([NCLS, BT], F32, name="ob", tag="ob")
        nc.scalar.activation(
            ob[:, :], ps3[0:NCLS, 0:BT], AFT.Identity, bias=bias_ap(6, NCLS)
        )
        nc.sync.dma_start(out[:, t * BT : (t + 1) * BT], ob[:, :])

    for it in range(NBT + 2):
        if it < NBT:
            fc1(it)
        if 0 <= it - 1 < NBT:
            fc2(it - 1)
        if 0 <= it - 2 < NBT:
            fc3(it - 2)


def _build_w1p(conv_w: np.ndarray, W1: np.ndarray) -> np.ndarray:
    """Fold the 3x3 valid conv into fc1: W1p[p, :] = sum over taps landing
    on pixel p of conv_w[dy,dx] * W1[q(p,dy,dx), :]. Zero-FLOP scatter-add."""
    w1p = np.zeros((NPIX, HID), np.float32)
    oi = np.arange(OHW)
    oj = np.arange(OHW)
    q = (oi[:, None] * OHW + oj[None, :]).ravel()
    for dy in range(3):
        for dx in range(3):
            p = ((oi[:, None] + dy) * IMG + (oj[None, :] + dx)).ravel()
            np.add.at(w1p, p, conv_w[dy, dx] * W1[q, :])
    return w1p


_NC_CACHE: list = []


def _get_nc():
    if _NC_CACHE:
        return _NC_CACHE[0]
    nc = bass.Bass("TRN2", target_bir_lowering=False, debug=False)
    xt = nc.dram_tensor("xt", [NPIX, B_CORE], BF16, kind="ExternalInput").ap()
    wp = nc.dram_tensor("wp", [128, WP_COLS], BF16, kind="ExternalInput").ap()
    bp = nc.dram_tensor("bp", [128, 7], F32, kind="ExternalInput").ap()
    out = nc.dram_tensor("out", [NCLS, B_CORE], F32, kind="ExternalOutput").ap()
    with tile.TileContext(nc) as tc:
        with ExitStack() as ctx:
            _emit(ctx, tc, xt, wp, bp, out)
    _legalize_single_wait(nc)
    _NC_CACHE.append(nc)
    return nc


def _pack_weights(inputs: dict) -> tuple:
    bf = ml_dtypes.bfloat16
    w1p = _build_w1p(
        np.asarray(inputs["conv_w"], dtype=np.float32),
        np.asarray(inputs["W1"], dtype=np.float32),
    )
    w2 = np.asarray(inputs["W2"], np.float32)
    w3 = np.asarray(inputs["W3"], np.float32)
    wp = np.zeros((128, WP_COLS), bf)
    # w1s: [112, 7, 300] pixel chunk-major
    wp[0:PC, 0:W1_COLS] = (
        w1p.reshape(NPC, PC, HID).transpose(1, 0, 2).reshape(PC, W1_COLS).astype(bf)
    )
    for hc, (h0, hp) in enumerate(H_CH):
        wp[0:hp, W2_OFF + hc * HID : W2_OFF + (hc + 1) * HID] = w2[
            h0 : h0 + hp, :
        ].astype(bf)
        wp[0:hp, W3_OFF + hc * NC_PAD : W3_OFF + hc * NC_PAD + NCLS] = w3[
            h0 : h0 + hp, :
        ].astype(bf)
    bpk = np.zeros((128, 7), np.float32)
    b1 = np.asarray(inputs["b1"], np.float32)
    b2 = np.asarray(inputs["b2"], np.float32)
    b3 = np.asarray(inputs["b3"], np.float32)
    for hc, (h0, hp) in enumerate(H_CH):
        bpk[0:hp, hc] = b1[h0 : h0 + hp]
        bpk[0:hp, 3 + hc] = b2[h0 : h0 + hp]
    bpk[0:NCLS, 6] = b3
    return wp, bpk


def _in_maps(inputs: dict) -> list:
    x = np.asarray(inputs["x"], dtype=np.float32)
    assert x.shape == (B_FULL, NPIX), x.shape
    wp, bpk = _pack_weights(inputs)
    bf = ml_dtypes.bfloat16
    common = {"wp": wp, "bp": bpk}
    xr = x.reshape(N_CORES, B_CORE, NPIX)
    return [
        {"xt": np.ascontiguousarray(xr[c].T.astype(bf)), **common}
        for c in range(N_CORES)
    ]


def kernel(**inputs) -> np.ndarray:
    nc = _get_nc()
    res = run_bass_kernel_spmd(nc, _in_maps(inputs), list(range(N_CORES)))
    return np.concatenate(
        [np.ascontiguousarray(res.results[c]["out"].T) for c in range(N_CORES)],
        axis=0,
    )


if __name__ == "__main__":
    rng = np.random.default_rng(0)
    ins = {
        "x": rng.standard_normal((B_FULL, NPIX), dtype=np.float32),
        "conv_w": rng.standard_normal((3, 3), dtype=np.float32) * 0.1,
        "W1": rng.standard_normal((FLAT, HID), dtype=np.float32) * 0.04,
        "b1": np.zeros(HID, np.float32),
        "W2": rng.standard_normal((HID, HID), dtype=np.float32) * 0.06,
        "b2": np.zeros(HID, np.float32),
        "W3": rng.standard_normal((HID, NCLS), dtype=np.float32) * 0.06,
        "b3": np.zeros(NCLS, np.float32),
    }
    y = kernel(**ins)
    from numpy.lib.stride_tricks import sliding_window_view

    img = ins["x"].reshape(-1, IMG, IMG)
    win = sliding_window_view(img, (3, 3), axis=(1, 2))
    conv = np.einsum("bijkl,kl->bij", win, ins["conv_w"]).reshape(-1, FLAT)
    h = np.maximum(conv @ ins["W1"] + ins["b1"], 0)
    h = np.maximum(h @ ins["W2"] + ins["b2"], 0)
    ref = h @ ins["W3"] + ins["b3"]
    err = np.abs(y - ref).max() / (np.abs(ref).max() + 1e-9)
    print("max rel err vs numpy:", err)
